# revision 28
# baseline (speedup 1.0000x reference)
"""Multi-head attention (B=2, S=2048, D=1024, H=16) on 8 NeuronCores.

Sharding: tensor-parallel over heads — 2 heads per core. Each core computes
q/k/v projections for its 128 output columns, full attention for its 2 heads
(both batches), and a partial out-projection [4096, 1024] in bf16. Host sums
the 8 partials in f64 and adds the output bias.

Device-side structure (v2 — single ACT-paced pipeline):
  - The kernel is paced by the scalar engine's exp over the 16.8M score
    elements per core (the hard floor: ~(N+352)/1.2GHz per tile). Everything
    else is scheduled into the PE/DVE/DMA slack of that pipeline.
  - Scores are computed per (batch, 512-wide q-slice) into mixed-head PSUM
    tiles [128 keys, h0 512 | h1 512] by PAIRS of row-tiled matmuls
    (contract=64 each, tile_position (0,0)/(64,0) auto-derived from base
    partitions) that run CONCURRENTLY in the two halves of the PE array.
  - Only K-proj(b0) + Q-proj(first 512 tokens) run before the attention
    pipeline starts. All remaining Q/K projection blocks, the whole V
    projection, the previous unit's attn@V + softmax-normalize, and the
    out-projection drain as budgeted filler work between exp tiles
    (express lane for producer-critical blocks, background lane for the
    rest; fillers only yield at PSUM accumulation-group boundaries).
  - V carries an all-ones column so attn@[V|1] also yields the softmax
    denominator; denominators go through reciprocal_approx_fast (custom
    DVE op, ~5x faster than InstReciprocal, 18-bit accurate) ->
    GpSimd partition_broadcast -> DVE multiply during PSUM evacuation.
  - softmax skips max-subtraction: scores are ~N(0, 0.33^2) by construction.
  - weights / xT are host-permuted so every DMA is >=1KB-contiguous rows.
  - output partials are written in bf16 (adds ~0.1% error, halves out DMA).
"""

import collections
import os

import numpy as np
import ml_dtypes

B, S, D, H = 2, 2048, 1024, 16
HD = D // H          # 64
BS = B * S           # 4096 tokens
NCORES = 8
HPC = H // NCORES    # heads per core = 2
CPC = HPC * HD       # output cols per core = 128
KC = D // 128        # contract chunks = 8
SB = 512             # q-slice width / projection sub-block width
NSB = BS // SB       # 8 token sub-blocks
NKT = S // 128       # 16 key tiles per batch
NQS = S // SB        # 4 q-slices per batch

BF16 = ml_dtypes.bfloat16

_prog = None


def _build_program():
    import concourse.bacc as bacc
    import concourse.tile as tile
    from concourse import mybir

    f32 = mybir.dt.float32
    bf16 = mybir.dt.bfloat16
    AF = mybir.ActivationFunctionType

    nc = bacc.Bacc("TRN2", debug=False, enable_asserts=False, num_devices=NCORES)

    xT = nc.dram_tensor("xT", [128, KC, BS], bf16, kind="ExternalInput").ap()
    wq = nc.dram_tensor("wq", [128, KC, CPC], bf16, kind="ExternalInput").ap()
    wk = nc.dram_tensor("wk", [128, KC, CPC], bf16, kind="ExternalInput").ap()
    wv = nc.dram_tensor("wv", [128, KC, CPC], bf16, kind="ExternalInput").ap()
    wo = nc.dram_tensor("wo", [CPC, D], bf16, kind="ExternalInput").ap()
    bq = nc.dram_tensor("bq", [CPC, 1], f32, kind="ExternalInput").ap()
    bk = nc.dram_tensor("bk", [CPC, 1], f32, kind="ExternalInput").ap()
    bv = nc.dram_tensor("bv", [1, CPC], bf16, kind="ExternalInput").ap()
    out = nc.dram_tensor("out", [BS, D], bf16, kind="ExternalOutput").ap()

    SCALE = float(1.0 / np.sqrt(HD))

    with tile.TileContext(nc) as tc:
        with (
            tc.tile_pool(name="big", bufs=1) as big,
            tc.tile_pool(name="sm", bufs=1) as sm,
            tc.tile_pool(name="attn", bufs=2) as attn,
            tc.tile_pool(name="etp", bufs=2) as etp,
            tc.tile_pool(name="ostage", bufs=4) as ostage,
            tc.tile_pool(name="ps", bufs=2, space="PSUM") as ps,
        ):
            # ---- resident SBUF tensors ----
            xt_sb = big.tile([128, KC, BS], bf16, name="xt_sb", tag="xt")
            qt_sb = big.tile([128, BS], bf16, name="qt_sb", tag="qt")
            kt_sb = big.tile([128, BS], bf16, name="kt_sb", tag="kt")
            # V|ones per head: [keys(128) x keytile(32) x (64 V + 1 ones)*2]
            v_sb = big.tile([128, B * NKT, 2 * (HD + 1)], bf16, name="v_sb", tag="v")
            wo_sb = big.tile([128, D], bf16, name="wo_sb", tag="wo")

            wq_sb = sm.tile([128, KC, CPC], bf16, name="wq_sb", tag="wq")
            wk_sb = sm.tile([128, KC, CPC], bf16, name="wk_sb", tag="wk")
            wv_sb = sm.tile([128, KC, CPC], bf16, name="wv_sb", tag="wv")
            bq_sb = sm.tile([CPC, 1], f32, name="bq_sb", tag="bq")
            bk_sb = sm.tile([CPC, 1], f32, name="bk_sb", tag="bk")
            bv_sb = sm.tile([1, CPC], bf16, name="bv_sb", tag="bv")
            ones_bf = sm.tile([1, 128], bf16, name="ones_bf", tag="onesb")

            nc.vector.memset(ones_bf, 1.0)
            nc.vector.memset(v_sb[:, :, HD : HD + 1], 1.0)
            nc.vector.memset(v_sb[:, :, 2 * HD + 1 : 2 * HD + 2], 1.0)

            # ---- DMAs, in consumption order ----
            nc.sync.dma_start(out=wk_sb, in_=wk)
            nc.sync.dma_start(out=wq_sb, in_=wq)
            nc.sync.dma_start(out=bk_sb, in_=bk)
            nc.sync.dma_start(out=bq_sb, in_=bq)
            # first K-proj/Q-proj sub-block needs tokens 0:512 of every chunk;
            # the rest arrives as a few large block transfers
            # dma_starts serialize per issuing engine (~0.6us trigger +
            # transfer each), so batch 1 (b0 tokens, prefix-critical) goes on
            # Sync in 2KB-contiguous 256KB chunks while batch 1's tokens
            # stream in parallel from the otherwise-idle GpSimd queue.
            for c in range(KC):
                nc.sync.dma_start(out=xt_sb[:, c, 0:1024], in_=xT[:, c, 0:1024])
            for c in range(KC):
                nc.gpsimd.dma_start(
                    out=xt_sb[:, c, 2048:4096], in_=xT[:, c, 2048:4096]
                )
            for c in range(KC):
                nc.sync.dma_start(out=xt_sb[:, c, 1024:2048], in_=xT[:, c, 1024:2048])
            nc.sync.dma_start(out=wv_sb, in_=wv)
            nc.sync.dma_start(out=bv_sb, in_=bv)
            nc.sync.dma_start(out=wo_sb, in_=wo)

            # ---- projection building blocks ----
            def qkproj_blk(name, w_sb, b_sb, dst, blk):
                """Project 1024 tokens -> dst[:, blk*1024:(blk+1)*1024]
                (transposed layout: head cols on partitions), as two atomic
                8-MM groups through 1-bank PSUM tiles."""
                for half in range(2):
                    t0 = blk * 1024 + half * 512
                    pp = ps.tile(
                        [128, 512], f32, name=f"pp_{name}{blk}{half}", tag="small", bufs=4
                    )
                    # two 4-MM chunks accumulating into one PSUM tile, with a
                    # yield between: a single 1.7us chunk would outrun the
                    # 2-buffer score pipeline and bubble the ACT engine
                    for c in range(KC):
                        if c == KC // 2:
                            yield 1.0
                        nc.tensor.matmul(
                            pp,
                            lhsT=w_sb[:, c, :],
                            rhs=xt_sb[:, c, t0 : t0 + 512],
                            start=(c == 0),
                            stop=(c == KC - 1),
                            skip_group_check=(c >= KC // 2),
                        )
                    nc.vector.tensor_scalar_add(dst[:, t0 : t0 + 512], pp, b_sb)
                    yield 1.0

            def vproj(kt):
                vp = ps.tile([128, CPC], f32, name=f"vp{kt}", tag="small", bufs=4)
                for c in range(KC):
                    nc.tensor.matmul(
                        vp,
                        lhsT=xt_sb[:, c, kt * 128 : (kt + 1) * 128],
                        rhs=wv_sb[:, c, :],
                        start=(c == 0),
                        stop=False,
                    )
                nc.tensor.matmul(vp, lhsT=ones_bf, rhs=bv_sb, start=False, stop=True)
                nc.vector.tensor_copy(
                    v_sb[:, kt, :].rearrange("p (h c) -> p h c", h=2)[:, :, 0:HD],
                    vp.rearrange("p (h c) -> p h c", h=2),
                )

            def gen_vproj(b, kt0=0, nkt=NKT):
                for kti in range(kt0, kt0 + nkt):
                    vproj(b * NKT + kti)
                    yield 0.8

            # ---- filler machinery: express preempts background ----
            express_q = collections.deque()
            bg_q = collections.deque()

            class Gen:
                def __init__(self, g):
                    self.g = g
                    self.done = False

                def step(self):
                    try:
                        return next(self.g)
                    except StopIteration:
                        self.done = True
                        return None

            def drain(budget):
                spent = 0.0
                while spent < budget:
                    q = express_q if express_q else bg_q
                    if not q:
                        return
                    v = q[0].step()
                    if v is None:
                        q.popleft()
                    else:
                        spent += v

            def drain_until(gen):
                """Emit queued fillers until `gen` (in a queue) is exhausted."""
                while not gen.done and (express_q or bg_q):
                    drain(10.0)

            ot_tiles = {}
            for b in range(B):
                ot_tiles[b] = attn.tile([128, S], bf16, name=f"ot{b}", tag="ot")

            def gen_attnv(b, h, qg, et):
                """attn@V + softmax-normalize for unit (b, h, qg), consuming
                exp tile et. Emitted as fillers during the next unit."""
                ot_sb = ot_tiles[b]
                hp = h * HD
                for qc in range(2):
                    op = ps.tile(
                        [HD + 1, 512], f32, name=f"op{b}{h}{qg}{qc}", tag="small", bufs=4
                    )
                    # two 8-MM chunks accumulating into one PSUM tile (see
                    # qkproj_blk: yields between chunks keep the ACT fed)
                    for kt in range(NKT):
                        if kt == NKT // 2:
                            yield 2.2
                        nc.tensor.matmul(
                            op,
                            lhsT=v_sb[:, b * NKT + kt, h * (HD + 1) : (h + 1) * (HD + 1)],
                            rhs=et[:, kt, qc * 512 : (qc + 1) * 512],
                            start=(kt == 0),
                            stop=(kt == NKT - 1),
                            skip_group_check=(kt >= NKT // 2),
                        )
                    # Z-chain emitted contiguously: PSUM buf rotation must not
                    # hand op's bank to a new writer before these reads exist.
                    # reciprocal_approx_fast needs its input staged to SBUF
                    # (the custom-DVE uop misreads PSUM operands).
                    zr = ostage.tile([1, 512], f32, name=f"zr{b}{h}{qg}{qc}", tag="zr", bufs=2)
                    nc.vector.tensor_copy(zr, op[HD : HD + 1, :])
                    rc = ostage.tile([1, 512], f32, name=f"rc{b}{h}{qg}{qc}", tag="rc", bufs=2)
                    nc.vector.reciprocal_approx_fast(rc, zr)
                    rbs = ostage.tile(
                        [HD, 512], f32, name=f"rbs{b}{h}{qg}{qc}", tag="rbs", bufs=2
                    )
                    nc.gpsimd.partition_broadcast(rbs, rc)
                    nc.vector.tensor_mul(
                        ot_sb[hp : hp + HD, qg * 1024 + qc * 512 : qg * 1024 + (qc + 1) * 512],
                        op[0:HD, :],
                        rbs,
                    )
                    yield 2.5

            def gen_outproj(b, qt0=0, nqt=S // 128, tail=False):
                ot_sb = ot_tiles[b]
                for qt in range(qt0, qt0 + nqt):
                    os_ = ostage.tile(
                        [128, 1024], bf16, name=f"os{b}{qt}", tag="os", bufs=4
                    )
                    for half in range(2):
                        pq = ps.tile(
                            [128, 512], f32, name=f"pq{b}{qt}{half}", tag="small", bufs=4
                        )
                        nc.tensor.matmul(
                            pq,
                            lhsT=ot_sb[:, qt * 128 : (qt + 1) * 128],
                            rhs=wo_sb[:, half * 512 : (half + 1) * 512],
                            start=True,
                            stop=True,
                        )
                        if tail and half == 0:
                            nc.scalar.copy(os_[:, half * 512 : (half + 1) * 512], pq)
                        else:
                            nc.vector.tensor_copy(
                                os_[:, half * 512 : (half + 1) * 512], pq
                            )
                    nc.sync.dma_start(
                        out=out[b * S + qt * 128 : b * S + (qt + 1) * 128, :],
                        in_=os_,
                    )
                    yield 1.8

            # scratch for the row-group serializer matmul (see below)
            ser_ps = ps.tile([1, 1], f32, name="ser_ps", tag="small", bufs=4)

            def serialize_pe():
                # A full-128-partition matmul conflicts with both 64-row
                # groups, forcing in-flight 64-contract matmuls to complete.
                # Guards head transitions: two adjacent independent matmuls
                # in different row groups run concurrently, and concurrent
                # row tiles corrupt interleaved PSUM cachelines.
                nc.tensor.matmul(
                    ser_ps, lhsT=kt_sb[:, 0:1], rhs=kt_sb[:, 0:1], start=True, stop=True
                )

            # ---- HAM warmup: ~6us of junk matmuls so the PE clock gate is
            # released (K=8/8, 2.4GHz) and stays released until the first
            # projection data lands. They read not-yet-written SBUF
            # (harmless; ser_ps is never read).
            for _ in range(100):
                nc.tensor.matmul(
                    ser_ps, lhsT=kt_sb[:, 0:1], rhs=kt_sb[:, 0:1], start=True, stop=True
                )

            # ---- minimal serial prefix: K-proj(b0, tokens 0:512) +
            # Q-proj(blk0); K-proj's second half joins the express lane ----
            gk0 = Gen(qkproj_blk("k", wk_sb, bk_sb, kt_sb, 0))
            gk0.step()
            for _ in qkproj_blk("q", wq_sb, bq_sb, qt_sb, 0):
                pass

            express_q.append(gk0)  # K blk0 half1 (keys 512:1024, by u0 kt4)
            express_q.append(Gen(qkproj_blk("k", wk_sb, bk_sb, kt_sb, 1)))
            express_q.append(Gen(qkproj_blk("q", wq_sb, bq_sb, qt_sb, 1)))
            bg_q.append(Gen(gen_vproj(0, 0, 8)))

            # unit u = (b, h, qg): q-block index = b*2 + qg, keys = batch b
            express_sched = {
                2: [("k", 2)],
                3: [("q", 2), ("k", 3)],
                4: [("q", 3)],
            }
            # pre entries enqueue BEFORE attnv(u-1) (they produce its inputs:
            # v tiles); post entries AFTER (they consume its ot outputs).
            # Loads are balanced so attn@V's atomic 16-MM chunks never spill
            # past their unit (a spill stalls the next unit's exps via the
            # double-buffered et pool).
            bg_pre_sched = {
                1: [(0, 8, 8), (1, 0, 8)],
                2: [(1, 8, 8)],
            }
            bg_post_sched = {
                4: [(0, 0, 4)],
                5: [(0, 4, 4), (0, 8, 2)],
                6: [(0, 10, 2), (0, 12, 4)],
                7: [(1, 0, 8)],
            }

            # ---- ACT-paced attention pipeline ----
            attnv_gens = {}
            units = [(b, h, qg) for b in range(B) for h in range(HPC) for qg in range(2)]
            for u, (b, h, qg) in enumerate(units):
                for kind, blk in express_sched.get(u, []):
                    if kind == "q":
                        express_q.append(Gen(qkproj_blk("q", wq_sb, bq_sb, qt_sb, blk)))
                    else:
                        express_q.append(Gen(qkproj_blk("k", wk_sb, bk_sb, kt_sb, blk)))
                for item in bg_pre_sched.get(u, []):
                    bg_q.append(Gen(gen_vproj(*item)))
                if u - 1 in attnv_gens:
                    bg_q.append(attnv_gens[u - 1])
                for item in bg_post_sched.get(u, []):
                    bg_q.append(Gen(gen_outproj(*item)))

                # et pool is double-buffered: unit u reuses unit u-2's buffer,
                # whose readers are attnv(u-2) — force-emit them first.
                if u - 2 in attnv_gens:
                    drain_until(attnv_gens[u - 2])

                if u > 0 and units[u - 1][1] != h:
                    serialize_pe()

                hp = h * HD
                qoff = b * S + qg * 1024
                et = etp.tile([128, NKT, 1024], bf16, name=f"et{u}", tag="et")
                for kt in range(NKT):
                    koff = b * S + kt * 128
                    sp = ps.tile([128, 1024], f32, name=f"sp{u}{kt}", tag="sp")
                    for qh in range(2):
                        nc.tensor.matmul(
                            sp[:, qh * 512 : (qh + 1) * 512],
                            lhsT=kt_sb[hp : hp + HD, koff : koff + 128],
                            rhs=qt_sb[hp : hp + HD, qoff + qh * 512 : qoff + (qh + 1) * 512],
                            start=True,
                            stop=True,
                        )
                    nc.scalar.activation(et[:, kt, :], sp, AF.Exp, scale=SCALE)
                    # no fillers while the exp pipeline primes: even one
                    # drained 3.4us attn@V chunk at the unit boundary stalls
                    # the next unit's first scores and bubbles the ACT.
                    if kt >= 3:
                        drain(1.78)
                attnv_gens[u] = Gen(gen_attnv(b, h, qg, et))

            # ---- tail ----
            bg_q.append(attnv_gens[len(units) - 1])
            bg_q.append(Gen(gen_outproj(1, 8, 8, tail=True)))
            drain(1e9)

    nc.compile()
    return nc


def _get_prog():
    global _prog
    if _prog is None:
        _prog = _build_program()
    return _prog


def _perm_ckc(a):
    """[D, N] -> [128, KC, N] with partition-contiguous rows."""
    return np.ascontiguousarray(a.reshape(KC, 128, -1).transpose(1, 0, 2))


def kernel(x, Wq, bq, Wk, bk, Wv, bv, Wo, bo):
    from concourse import bass_utils

    nc = _get_prog()

    xT = np.asarray(x, dtype=np.float32).reshape(BS, D).T.astype(BF16)
    xTp = _perm_ckc(xT)

    in_maps = []
    for c in range(NCORES):
        cols = slice(c * CPC, (c + 1) * CPC)
        in_maps.append(
            {
                "xT": xTp,
                "wq": _perm_ckc(Wq[cols, :].T.astype(BF16)),
                "wk": _perm_ckc(Wk[cols, :].T.astype(BF16)),
                "wv": _perm_ckc(Wv[cols, :].T.astype(BF16)),
                "wo": np.ascontiguousarray(Wo[:, cols].T).astype(BF16),
                "bq": np.asarray(bq[cols], np.float32).reshape(CPC, 1),
                "bk": np.asarray(bk[cols], np.float32).reshape(CPC, 1),
                "bv": np.asarray(bv[cols], np.float32).reshape(1, CPC).astype(BF16),
            }
        )

    res = bass_utils.run_bass_kernel_spmd(
        nc,
        in_maps,
        core_ids=list(range(NCORES)),
        trace=bool(int(os.environ.get("KERNEL_TRACE", "0"))),
    )
    kernel.last_results = res

    acc = np.zeros((BS, D), np.float64)
    for c in range(NCORES):
        acc += res.results[c]["out"].astype(np.float64)
    acc += np.asarray(bo, np.float64)[None, :]
    return acc.reshape(B, S, D).astype(np.float32)
